# revision 1
# baseline (speedup 1.0000x reference)
import numpy as np

B, S, H, L, W, NHW = 32, 512, 512, 2, 4, 2
NCORES = 8
BL = B // NCORES
P = 128
HB = H // P
MB = 2 * H // P
SW = S + W

_CACHE = {}


def _build_nc():
    import concourse.bass as bass
    import concourse.tile as tile
    from concourse import bacc, mybir

    f32 = mybir.dt.float32
    f32r = mybir.dt.float32r
    AF = mybir.ActivationFunctionType

    nc = bacc.Bacc("TRN2", target_bir_lowering=False)

    xt = nc.dram_tensor("xt", [BL, H, S], f32r, kind="ExternalInput")
    wt = nc.dram_tensor("wt", [L, 2, NHW, H, 2 * H], f32r, kind="ExternalInput")
    ids = nc.dram_tensor("ids", [L, 2, W + 1, P, P], f32r, kind="ExternalInput")
    padl = nc.dram_tensor("padl", [L, H, W], f32r, kind="ExternalInput")
    padr = nc.dram_tensor("padr", [L, H, W], f32r, kind="ExternalInput")
    hwb = nc.dram_tensor("hwb", [L, 2, NHW, P, MB], f32, kind="ExternalInput")
    ws = nc.dram_tensor("ws", [L, 2, W + 1], f32, kind="ExternalInput")
    out = nc.dram_tensor("out", [L, BL, 2 * H, S], f32, kind="ExternalOutput")

    with tile.TileContext(nc) as tc:
        with (
            tc.tile_pool(name="state", bufs=1) as state_pool,
            tc.tile_pool(name="singles", bufs=1) as singles,
            tc.tile_pool(name="ids", bufs=2) as ids_pool,
            tc.tile_pool(name="wt", bufs=2) as wt_pool,
            tc.tile_pool(name="evac", bufs=2) as evac_pool,
            tc.tile_pool(name="ps", bufs=2, space="PSUM") as ps_pool,
        ):
            hwb_sb = singles.tile([P, L, 2, NHW, MB], f32, tag="hwb", name="hwb_sb")
            nc.sync.dma_start(
                out=hwb_sb, in_=hwb.rearrange("l d i p m -> p l d i m")
            )
            ws_sb = singles.tile([P, L, 2, W + 1], f32, tag="ws", name="ws_sb")
            wsap = ws[:]
            nc.sync.dma_start(
                out=ws_sb,
                in_=bass.AP(tensor=wsap.tensor, offset=wsap.offset,
                            ap=[[0, P]] + list(wsap.ap)),
            )

            bufs = {
                0: [state_pool.tile([P, HB, BL, SW], f32r, tag="fA", name="fA"),
                    state_pool.tile([P, HB, BL, SW], f32r, tag="fB", name="fB")],
                1: [state_pool.tile([P, HB, BL, SW], f32r, tag="bA", name="bA"),
                    state_pool.tile([P, HB, BL, SW], f32r, tag="bB", name="bB")],
            }
            OFF = {0: W, 1: 0}
            PADOFF = {0: 0, 1: S}

            for hb in range(HB):
                xv = xt[:, hb * P:(hb + 1) * P, :].rearrange("b p s -> p b s")
                for d in range(2):
                    o = OFF[d]
                    nc.sync.dma_start(
                        out=bufs[d][0][:, hb, :, o:o + S], in_=xv
                    )

            DVE_SPLIT = {0: 0, 1: 0, 2: 1, 3: BL}

            def conv(l, d, ids_sb, src, dst):
                o = OFF[d]
                for hb in range(HB):
                    nb = BL - DVE_SPLIT[hb]
                    for b in range(nb, BL):
                        acc = dst[:, hb, b, o:o + S]
                        acc32 = acc.bitcast(f32)
                        nc.vector.tensor_scalar_mul(
                            acc, src[:, hb, b, 0:S].bitcast(f32),
                            ws_sb[:, l, d, 0:1],
                        )
                        for k in range(1, W + 1):
                            nc.vector.scalar_tensor_tensor(
                                acc,
                                src[:, hb, b, k:k + S].bitcast(f32),
                                ws_sb[:, l, d, k:k + 1],
                                acc32,
                                op0=mybir.AluOpType.mult,
                                op1=mybir.AluOpType.add,
                            )
                    if nb == 0:
                        continue
                    ps = ps_pool.tile([P, BL, S], f32, tag="ps", name="ps")
                    for k in range(W + 1):
                        for b in range(nb):
                            nc.tensor.matmul(
                                ps[:, b, :],
                                lhsT=ids_sb[:, k, :],
                                rhs=src[:, hb, b, k:k + S],
                                start=(k == 0),
                                stop=(k == W),
                            )
                    nc.scalar.copy(
                        out=dst[:, hb, :nb, o:o + S], in_=ps[:, :nb, :]
                    )

            def hw_linear(l, d, i, xin, xout):
                o = OFF[d]
                wt_sb = wt_pool.tile([P, HB, 2 * H], f32r, tag="wt", name="wt_sb")
                nc.sync.dma_start(
                    out=wt_sb,
                    in_=wt[l, d, i].rearrange("(kb p) o -> p kb o", p=P),
                )
                for j in range(HB):
                    nl = evac_pool.tile([P, BL, S], f32, tag="nl", name="nl")
                    g = evac_pool.tile([P, BL, S], f32, tag="g", name="g")
                    for half, (dst, fn) in enumerate(
                        ((nl, AF.Relu), (g, AF.Sigmoid))
                    ):
                        mb = j + HB * half
                        ps = ps_pool.tile([P, BL, S], f32, tag="ps", name="ps")
                        for kb in range(HB):
                            for b in range(BL):
                                nc.tensor.matmul(
                                    ps[:, b, :],
                                    lhsT=wt_sb[:, kb, mb * P:(mb + 1) * P],
                                    rhs=xin[:, kb, b, o:o + S],
                                    start=(kb == 0),
                                    stop=(kb == HB - 1),
                                )
                        nc.scalar.activation(
                            out=dst,
                            in_=ps[:],
                            func=fn,
                            bias=hwb_sb[:, l, d, i, mb:mb + 1],
                        )
                    xi32 = xin[:, j, :, o:o + S].bitcast(f32)
                    xoj = xout[:, j, :, o:o + S]
                    xo32 = xoj.bitcast(f32)
                    nc.vector.tensor_sub(xoj, xi32, nl)
                    nc.vector.tensor_mul(xoj, g, xo32)
                    nc.vector.tensor_add(xoj, xo32, nl)

            for l in range(L):
                ids_sb = {}
                for d in range(2):
                    ids_sb[d] = ids_pool.tile(
                        [P, W + 1, P], f32r, tag="ids", name="ids_sb"
                    )
                    nc.sync.dma_start(
                        out=ids_sb[d],
                        in_=ids[l, d].rearrange("k p m -> p k m"),
                    )
                    psrc = (padl if d == 0 else padr)[l].rearrange(
                        "(hb p) w -> p hb w", p=P
                    )
                    po = PADOFF[d]
                    for b in range(BL):
                        nc.sync.dma_start(
                            out=bufs[d][0][:, :, b, po:po + W], in_=psrc
                        )
                for d in range(2):
                    p, q = bufs[d]
                    conv(l, d, ids_sb[d], p, q)
                for i in range(NHW):
                    for d in range(2):
                        p, q = bufs[d]
                        if i == 0:
                            hw_linear(l, d, i, q, p)
                        else:
                            hw_linear(l, d, i, p, q)
                for d in range(2):
                    p, q = bufs[d]
                    o = OFF[d]
                    hoff = 0 if d == 0 else H
                    for hb in range(HB):
                        for b in range(BL):
                            ov = out[l, b][hoff + hb * P:hoff + (hb + 1) * P, :]
                            nc.sync.dma_start(
                                out=ov,
                                in_=q[:, hb, b, o:o + S].bitcast(f32),
                            )
                    bufs[d] = [q, p]
    nc.finalize()
    return nc


def _get_nc():
    if "nc" not in _CACHE:
        _CACHE["nc"] = _build_nc()
    return _CACHE["nc"]


def _prep_shared(inputs):
    fwd_pads = np.asarray(inputs["fwd_pads"], np.float32)
    bwd_pads = np.asarray(inputs["bwd_pads"], np.float32)
    fwd_ws = np.asarray(inputs["fwd_ws"], np.float32)
    bwd_ws = np.asarray(inputs["bwd_ws"], np.float32)
    fwd_hw_W = np.asarray(inputs["fwd_hw_W"], np.float32)
    fwd_hw_b = np.asarray(inputs["fwd_hw_b"], np.float32)
    bwd_hw_W = np.asarray(inputs["bwd_hw_W"], np.float32)
    bwd_hw_b = np.asarray(inputs["bwd_hw_b"], np.float32)

    wt = np.empty((L, 2, NHW, H, 2 * H), np.float32)
    hwb = np.empty((L, 2, NHW, P, MB), np.float32)
    for l in range(L):
        for i in range(NHW):
            wt[l, 0, i] = fwd_hw_W[l, i].T
            wt[l, 1, i] = bwd_hw_W[l, i].T
            hwb[l, 0, i] = fwd_hw_b[l, i].reshape(MB, P).T
            hwb[l, 1, i] = bwd_hw_b[l, i].reshape(MB, P).T

    eye = np.eye(P, dtype=np.float32)
    ids = np.empty((L, 2, W + 1, P, P), np.float32)
    for l in range(L):
        for k in range(W + 1):
            ids[l, 0, k] = fwd_ws[l, k] * eye
            ids[l, 1, k] = bwd_ws[l, k] * eye

    ws = np.stack([fwd_ws, bwd_ws], axis=1)

    return {
        "ws": np.ascontiguousarray(ws),
        "wt": np.ascontiguousarray(wt),
        "ids": np.ascontiguousarray(ids),
        "padl": np.ascontiguousarray(fwd_pads.transpose(0, 2, 1)),
        "padr": np.ascontiguousarray(bwd_pads.transpose(0, 2, 1)),
        "hwb": np.ascontiguousarray(hwb),
    }


def kernel(**inputs) -> np.ndarray:
    from concourse.bass_utils import run_bass_kernel_spmd

    x = np.asarray(inputs["inputs"], np.float32)
    xt = np.ascontiguousarray(x.transpose(0, 2, 1))
    shared = _prep_shared(inputs)

    nc = _get_nc()
    in_maps = []
    for c in range(NCORES):
        m = dict(shared)
        m["xt"] = np.ascontiguousarray(xt[c * BL:(c + 1) * BL])
        in_maps.append(m)
    res = run_bass_kernel_spmd(nc, in_maps, core_ids=list(range(NCORES)))
    _CACHE["last_res"] = res
    outs = [r["out"] for r in res.results]
    full = np.concatenate(outs, axis=1)
    return np.ascontiguousarray(full.transpose(0, 1, 3, 2))



# revision 47
# speedup vs baseline: 1.2908x; 1.2908x over previous
import numpy as np
import ml_dtypes

B, S, H, L, W, NHW = 32, 512, 512, 2, 4, 2
NCORES = 8
BL = B // NCORES
P = 128
HB = H // P
MB = 2 * H // P
SW = S + W

_CACHE = {}


def _build_nc():
    import concourse.bass as bass
    import concourse.tile as tile
    from concourse import bacc, mybir

    f32 = mybir.dt.float32
    bf16 = mybir.dt.bfloat16
    AF = mybir.ActivationFunctionType
    ALU = mybir.AluOpType

    nc = bacc.Bacc("TRN2", target_bir_lowering=False)

    xf = nc.dram_tensor("xf", [BL, H, SW], bf16, kind="ExternalInput")
    xb = nc.dram_tensor("xb", [BL, H, SW], bf16, kind="ExternalInput")
    wt = nc.dram_tensor("wt", [L, 2, NHW, H, 2 * H], bf16, kind="ExternalInput")
    padl = nc.dram_tensor("padl", [L, H, W], bf16, kind="ExternalInput")
    padr = nc.dram_tensor("padr", [L, H, W], bf16, kind="ExternalInput")
    hwb = nc.dram_tensor("hwb", [L, 2, NHW, P, MB], f32, kind="ExternalInput")
    ws = nc.dram_tensor("ws", [L, 2, W + 1], f32, kind="ExternalInput")
    ids = nc.dram_tensor("ids", [W + 1, P, P], bf16, kind="ExternalInput")
    out = nc.dram_tensor("out", [L, BL, 2 * H, S], bf16, kind="ExternalOutput")

    with tile.TileContext(nc) as tc:
        with (
            tc.tile_pool(name="state", bufs=1) as state_pool,
            tc.tile_pool(name="singles", bufs=1) as singles,
            tc.tile_pool(name="wt", bufs=4) as wt_pool,
            tc.tile_pool(name="nlg", bufs=1) as nlg_pool,
            tc.tile_pool(name="convt", bufs=1) as convt_pool,
            tc.tile_pool(name="ps", bufs=8, space="PSUM") as ps_pool,
        ):
            ids_sb = singles.tile([P, W + 1, P], bf16, tag="ids",
                                  name="ids_sb")
            nc.sync.dma_start(
                out=ids_sb, in_=ids.rearrange("k p m -> p k m")
            )
            ws_sb = singles.tile([P, L, 2, W + 1], f32, tag="ws", name="ws_sb")
            wsap = ws[:]
            nc.sync.dma_start(
                out=ws_sb,
                in_=bass.AP(tensor=wsap.tensor, offset=wsap.offset,
                            ap=[[0, P]] + list(wsap.ap)),
            )
            hwb_sb = singles.tile([P, L, 2, NHW, MB], f32, tag="hwb",
                                  name="hwb_sb")

            bufs = {
                0: [state_pool.tile([P, HB, BL, SW], bf16, tag="fA", name="fA"),
                    state_pool.tile([P, HB, BL, SW], bf16, tag="fB", name="fB")],
                1: [state_pool.tile([P, HB, BL, SW], bf16, tag="bA", name="bA"),
                    state_pool.tile([P, HB, BL, SW], bf16, tag="bB", name="bB")],
            }
            OFF = {0: W, 1: 0}
            PADOFF = {0: 0, 1: S}

            dve_t = convt_pool.tile([P, 6, HB, S], bf16, tag="dvet",
                                    name="dve_t")
            pl_t = convt_pool.tile([P, 6, HB, S], bf16, tag="plt",
                                   name="pl_t")
            nl_t = nlg_pool.tile([P, HB, BL, S], bf16, tag="nl", name="nl_t")
            g_t = nlg_pool.tile([P, HB, BL, S], bf16, tag="g", name="g_t")

            def pads_dma(l, tgt):
                for d in range(2):
                    psrc = (padl if d == 0 else padr)[l].rearrange(
                        "(hb p) w -> p hb w", p=P
                    )
                    po = PADOFF[d]
                    for b in range(BL):
                        nc.sync.dma_start(
                            out=tgt[d][:, :, b, po:po + W], in_=psrc
                        )

            def conv_chain(l, d, b, src, dst):
                o = OFF[d]
                if b == BL - 1 and (l, d) in ((0, 1), (1, 0)):
                    def rw(k):
                        wap = ws_sb[:, l, d, k:k + 1]
                        return bass.AP(
                            tensor=wap.tensor, offset=wap.offset,
                            ap=[list(wap.ap[0]), [0, HB], [0, S]],
                        )
                    for k in range(W + 1):
                        nc.gpsimd.tensor_tensor(
                            pl_t[:, k], src[:, :, b, k:k + S], rw(k),
                            op=ALU.mult,
                        )
                    nc.gpsimd.tensor_add(pl_t[:, 5], pl_t[:, 0], pl_t[:, 1])
                    nc.gpsimd.tensor_add(pl_t[:, 0], pl_t[:, 2], pl_t[:, 3])
                    nc.gpsimd.tensor_add(pl_t[:, 1], pl_t[:, 5], pl_t[:, 0])
                    nc.gpsimd.tensor_add(
                        dst[:, :, b, o:o + S], pl_t[:, 1], pl_t[:, 4]
                    )
                    return
                wsl = lambda k: ws_sb[:, l, d, k:k + 1]
                for k in range(W + 1):
                    nc.vector.tensor_scalar(
                        dve_t[:, k], src[:, :, b, k:k + S], wsl(k), None,
                        op0=ALU.mult,
                    )
                nc.vector.tensor_add(dve_t[:, 5], dve_t[:, 0], dve_t[:, 1])
                nc.vector.tensor_add(dve_t[:, 0], dve_t[:, 2], dve_t[:, 3])
                nc.vector.tensor_add(dve_t[:, 1], dve_t[:, 5], dve_t[:, 0])
                nc.vector.tensor_add(
                    dst[:, :, b, o:o + S], dve_t[:, 1], dve_t[:, 4]
                )

            wts = {}

            def emit_wt(s):
                if s >= 4 * L or s in wts:
                    return
                l, r = divmod(s, 4)
                i, d = divmod(r, 2)
                wts[s] = wt_pool.tile(
                    [P, HB, 2 * H], bf16, tag="wt", name=f"wt{s}_sb"
                )
                nc.sync.dma_start(
                    out=wts[s],
                    in_=wt[l, d, i].rearrange("(kb p) o -> p kb o", p=P),
                )

            def hw_linear(l, d, i, xin, xout, fine_tail=False):
                o = OFF[d]
                hoff = 0 if d == 0 else H
                s = 4 * l + 2 * i + d
                emit_wt(s + 4)
                wt_sb = wts[s]
                last = i == NHW - 1
                for b in range(BL):
                    for j in range(HB):
                        pss = {}
                        for half, mb in ((0, j), (1, j + HB)):
                            ps = ps_pool.tile([P, S], f32, tag="ps",
                                              name="ps")
                            pss[half] = ps
                            for kb in range(HB):
                                nc.tensor.matmul(
                                    ps[:],
                                    lhsT=wt_sb[:, kb, mb * P:(mb + 1) * P],
                                    rhs=xin[:, kb, b, o:o + S],
                                    start=(kb == 0),
                                    stop=(kb == HB - 1),
                                )
                        nc.scalar.activation(
                            out=nl_t[:, j, b, :], in_=pss[0][:],
                            func=AF.Relu,
                            bias=hwb_sb[:, l, d, i, j:j + 1],
                        )
                        nc.scalar.activation(
                            out=g_t[:, j, b, :], in_=pss[1][:],
                            func=AF.Sigmoid,
                            bias=hwb_sb[:, l, d, i, j + HB:j + HB + 1],
                        )
                        if fine_tail:
                            xi = xin[:, j, b, o:o + S]
                            xo = xout[:, j, b, o:o + S]
                            nc.vector.tensor_sub(xo, xi, nl_t[:, j, b, :])
                            nc.vector.tensor_mul(xo, g_t[:, j, b, :], xo)
                            nc.vector.tensor_add(xo, xo, nl_t[:, j, b, :])
                            nc.sync.dma_start(
                                out=out[l, b,
                                        hoff + j * P:hoff + (j + 1) * P, :],
                                in_=xo,
                            )
                    if not fine_tail:
                        xi = xin[:, :, b, o:o + S]
                        xo = xout[:, :, b, o:o + S]
                        nc.vector.tensor_sub(xo, xi, nl_t[:, :, b, :])
                        nc.vector.tensor_mul(xo, g_t[:, :, b, :], xo)
                        nc.vector.tensor_add(xo, xo, nl_t[:, :, b, :])
                        if last:
                            ov = out[l, b, hoff:hoff + H, :].rearrange(
                                "(hb p) s -> p hb s", p=P
                            )
                            nc.sync.dma_start(
                                out=ov, in_=xout[:, :, b, o:o + S]
                            )
                if last and l + 1 < L:
                    conv_chain(l + 1, d, BL - 1, xout, bufs[d][0])
                    for b in range(BL - 1):
                        conv_chain(l + 1, d, b, xout, bufs[d][0])

            for b in range(2):
                xv = xf[b].rearrange("(hb p) s -> p hb s", p=P)
                nc.sync.dma_start(out=bufs[0][0][:, :, b, :], in_=xv)
            nc.sync.dma_start(
                out=hwb_sb, in_=hwb.rearrange("l d i p m -> p l d i m")
            )
            emit_wt(0)
            xv3 = xb[BL - 1].rearrange("(hb p) s -> p hb s", p=P)
            nc.sync.dma_start(out=bufs[1][0][:, :, BL - 1, :], in_=xv3)
            for b in range(2, BL):
                xv = xf[b].rearrange("(hb p) s -> p hb s", p=P)
                nc.sync.dma_start(out=bufs[0][0][:, :, b, :], in_=xv)
            for b in range(BL - 1):
                xv = xb[b].rearrange("(hb p) s -> p hb s", p=P)
                nc.sync.dma_start(out=bufs[1][0][:, :, b, :], in_=xv)
            emit_wt(1)
            for l in range(1, L):
                pads_dma(l, {d: bufs[d][l % 2] for d in range(2)})
            emit_wt(2)
            emit_wt(3)

            for l in range(L):
                if l == 0:
                    for b in range(2):
                        for hb in range(HB):
                            psc = ps_pool.tile([P, S], f32, tag="ps",
                                               name="psc")
                            for k in range(W + 1):
                                nc.tensor.matmul(
                                    psc[:],
                                    lhsT=ids_sb[:, k, :],
                                    rhs=bufs[0][0][:, hb, b, k:k + S],
                                    start=(k == 0),
                                    stop=(k == W),
                                )
                            nc.scalar.copy(
                                out=bufs[0][1][:, hb, b,
                                               OFF[0]:OFF[0] + S],
                                in_=psc[:],
                            )
                    conv_chain(0, 1, BL - 1, bufs[1][0], bufs[1][1])
                    for b in range(2, BL):
                        conv_chain(0, 0, b, bufs[0][0], bufs[0][1])
                    for b in range(BL - 1):
                        conv_chain(0, 1, b, bufs[1][0], bufs[1][1])
                for i in range(NHW):
                    for d in range(2):
                        p, q = bufs[d]
                        fine = (l == L - 1) and (i == NHW - 1)
                        if i == 0:
                            hw_linear(l, d, i, q, p, fine_tail=fine)
                        else:
                            hw_linear(l, d, i, p, q, fine_tail=fine)
                for d in range(2):
                    p, q = bufs[d]
                    bufs[d] = [q, p]
    nc.finalize()
    return nc


def _get_nc():
    if "nc" not in _CACHE:
        _CACHE["nc"] = _build_nc()
    return _CACHE["nc"]


def _prep_shared(inputs):
    bf = ml_dtypes.bfloat16
    fwd_pads = np.asarray(inputs["fwd_pads"], np.float32)
    bwd_pads = np.asarray(inputs["bwd_pads"], np.float32)
    fwd_ws = np.asarray(inputs["fwd_ws"], np.float32)
    bwd_ws = np.asarray(inputs["bwd_ws"], np.float32)
    fwd_hw_W = np.asarray(inputs["fwd_hw_W"], np.float32)
    fwd_hw_b = np.asarray(inputs["fwd_hw_b"], np.float32)
    bwd_hw_W = np.asarray(inputs["bwd_hw_W"], np.float32)
    bwd_hw_b = np.asarray(inputs["bwd_hw_b"], np.float32)

    wtv = np.empty((L, 2, NHW, H, 2 * H), np.float32)
    hwbv = np.empty((L, 2, NHW, P, MB), np.float32)
    for l in range(L):
        for i in range(NHW):
            wtv[l, 0, i] = fwd_hw_W[l, i].T
            wtv[l, 1, i] = bwd_hw_W[l, i].T
            hwbv[l, 0, i] = fwd_hw_b[l, i].reshape(MB, P).T
            hwbv[l, 1, i] = bwd_hw_b[l, i].reshape(MB, P).T

    wsv = np.stack([fwd_ws, bwd_ws], axis=1)
    eye = np.eye(P, dtype=np.float32)
    idsv = np.empty((W + 1, P, P), np.float32)
    for k in range(W + 1):
        idsv[k] = fwd_ws[0, k] * eye

    return {
        "ws": np.ascontiguousarray(wsv),
        "ids": np.ascontiguousarray(idsv).astype(bf),
        "wt": np.ascontiguousarray(wtv).astype(bf),
        "padl": np.ascontiguousarray(fwd_pads.transpose(0, 2, 1)).astype(bf),
        "padr": np.ascontiguousarray(bwd_pads.transpose(0, 2, 1)).astype(bf),
        "hwb": np.ascontiguousarray(hwbv),
    }


def kernel(**inputs) -> np.ndarray:
    from concourse.bass_utils import run_bass_kernel_spmd

    bf = ml_dtypes.bfloat16
    x = np.asarray(inputs["inputs"], np.float32)
    xt = x.transpose(0, 2, 1)
    pl0 = np.broadcast_to(
        np.asarray(inputs["fwd_pads"], np.float32)[0].T[None], (B, H, W)
    )
    pr0 = np.broadcast_to(
        np.asarray(inputs["bwd_pads"], np.float32)[0].T[None], (B, H, W)
    )
    xfv = np.ascontiguousarray(
        np.concatenate([pl0, xt], axis=2)).astype(bf)
    xbv = np.ascontiguousarray(
        np.concatenate([xt, pr0], axis=2)).astype(bf)
    shared = _prep_shared(inputs)

    nc = _get_nc()
    in_maps = []
    for c in range(NCORES):
        m = dict(shared)
        m["xf"] = np.ascontiguousarray(xfv[c * BL:(c + 1) * BL])
        m["xb"] = np.ascontiguousarray(xbv[c * BL:(c + 1) * BL])
        in_maps.append(m)
    res = run_bass_kernel_spmd(nc, in_maps, core_ids=list(range(NCORES)))
    _CACHE["last_res"] = res
    outs = [r["out"] for r in res.results]
    full = np.concatenate(outs, axis=1).astype(np.float32)
    return np.ascontiguousarray(full.transpose(0, 1, 3, 2))
